# revision 9
# baseline (speedup 1.0000x reference)
"""Trainium2 Bass kernel for nn_DGM_d (retrieval_knn).

Model: x = einsum(mean_t(x_pre), W); lq = pairwise_sq_dists(x[0]) * temp;
D2 = pairwise_sq_dists(lq); idx = top_k(-D2, 8); gather + logprobs; edges.

Device strategy (8 cores, query-row sharded):
  With U~ = [x0, s, 1] (n x 66) and the mixing matrix H (x0 block -> -2*I,
  s <-> 1 swap), lq = U~ H U~^T exactly (rank 66). The D2 row-ranking then
  reduces to A = 2*U~ W~ U~^T - 1 r^T with W~ = H (U~^T U~) H = Gram(U~ H)
  (66x66) and r_j = u~_j^T W~ u~_j — a 97-wide padded contraction instead
  of the naive 4096-wide one (~60x fewer flops).

  Component layout on partitions (legal partial-access starts 0/32/64/96):
  rows 0..63 x0^T, row 64 s, row 65 ones, rows 66..95 zero, row 96 -r.

  Each core computes A for its 512 query rows against all 4096 columns and
  takes top-8 per row with the HW max8/max_index instructions (semantics
  match jax.lax.top_k incl. tie order). Embeddings are computed on device
  (mean via DVE reduce + PE transpose + matmul); x0 shards are AllGathered.
  logprobs come from gathered neighbor embeddings (indirect DMA) so
  self-pairs give exactly 0, matching the reference's direct formula.
"""

import functools

import numpy as np

import concourse.bacc as bacc
import concourse.bass as bass
import concourse.mybir as mybir
from concourse import bass_utils
from concourse.masks import make_identity
from concourse.tile import TileContext

N = 4096          # nodes
D = 64            # embed dim
F = 32            # input feature dim
T = 24            # temporal dim
K = 8             # neighbors
NC = 8            # cores
SH = N // NC      # 512 rows per core
CP = 97           # padded contract dim: x0(64) + s + ones + pad(30) + r
RT = SH // 128    # 4 row-tiles per core
CH = N // 512     # 8 column chunks of 512
f32 = mybir.dt.float32
u32 = mybir.dt.uint32


def build_nc(temp: float):
    nc = bacc.Bacc("TRN2", target_bir_lowering=False, debug=False, num_devices=NC)

    xp = nc.dram_tensor("xp", [2, T, SH, F], f32, kind="ExternalInput")
    w_in = nc.dram_tensor("w_in", [F, D], f32, kind="ExternalInput")
    cones = nc.dram_tensor("cones", [N], f32, kind="ExternalInput")
    out_x0 = nc.dram_tensor("out_x0", [SH, D], f32, kind="ExternalOutput")
    out_x1 = nc.dram_tensor("out_x1", [SH, D], f32, kind="ExternalOutput")
    out_idx = nc.dram_tensor("out_idx", [SH, K], u32, kind="ExternalOutput")
    out_logp = nc.dram_tensor("out_logp", [SH, K], f32, kind="ExternalOutput")

    with TileContext(nc) as tc:
        with (
            tc.tile_pool(name="dram", bufs=1, space="DRAM") as dram,
            tc.tile_pool(name="persist", bufs=1) as ps,
            tc.tile_pool(name="work", bufs=2) as wk,
            tc.tile_pool(name="vch", bufs=3) as vchp,
            tc.tile_pool(name="abuf", bufs=2) as abp,
            tc.tile_pool(name="pp_mm", bufs=2, space="PSUM") as pmm,
            tc.tile_pool(name="pp_w66", bufs=1, space="PSUM") as pw66,
            tc.tile_pool(name="pp_tr", bufs=2, space="PSUM") as ptr,
            tc.tile_pool(name="pp_a", bufs=3, space="PSUM") as pa,
        ):
            # ---------- constants ----------
            ident = ps.tile([128, 128], f32)
            make_identity(nc, ident[:])
            w_sb = ps.tile([F, D], f32)
            nc.sync.dma_start(out=w_sb[:], in_=w_in[:])
            ones64 = ps.tile([D, 1], f32)
            nc.sync.dma_start(out=ones64[:], in_=cones[:D].rearrange("(p o) -> p o", o=1))
            ones97 = ps.tile([CP, 1], f32)
            nc.sync.dma_start(out=ones97[:], in_=cones[:CP].rearrange("(p o) -> p o", o=1))

            # ---------- phase A: load shard, mean over t, embed ----------
            # SBUF layout [p = n//4, t, q = n%4, f]; DRAM runs of 512B.
            x_meanT = []  # [F, SH] per batch: transposed means
            for b in range(2):
                ld = wk.tile([128, T, 4, F], f32, tag="ld")
                nc.sync.dma_start(
                    out=ld[:], in_=xp[b].rearrange("t (p q) f -> p t q f", q=4)
                )
                m = wk.tile([128, 4, F], f32, tag="m")
                nc.vector.reduce_sum(
                    out=m[:], in_=ld[:].rearrange("p t q f -> p q f t"),
                    axis=mybir.AxisListType.X,
                )
                mt_ps = ptr.tile([128, 128], f32)
                nc.tensor.transpose(
                    out=mt_ps[:], in_=m[:].rearrange("p q f -> p (q f)"), identity=ident[:]
                )
                xmt = wk.tile([F, SH], f32, tag="xmt")
                xmt_v = xmt[:].rearrange("f (n q) -> f n q", q=4)
                for q in range(4):
                    # scale by 1/T to turn the t-sum into the mean
                    nc.scalar.mul(xmt_v[:, :, q], mt_ps[q * F:(q + 1) * F, :], 1.0 / T)
                x_meanT.append(xmt)

            # u~_shard^T [CP, SH]: rows 0..63 x0T_shard, 64 s, 65 ones, rest 0
            ut_sh = ps.tile([CP, SH], f32)
            nc.vector.memset(ut_sh[D:96, :], 0.0)
            nc.vector.memset(ut_sh[D:D + 2, :], 1.0)  # row64 (s, for now) + row65
            nc.vector.memset(ut_sh[96:CP, :], 0.0)
            x0t_ps = pmm.tile([D, SH], f32, tag="mm")
            nc.tensor.matmul(out=x0t_ps[:], lhsT=w_sb[:], rhs=x_meanT[0][:],
                             start=True, stop=True)
            nc.scalar.copy(out=ut_sh[0:D, :], in_=x0t_ps[:])

            # natural-layout embeds for both batches: [128, RT, D]
            xnat = []
            for b in range(2):
                xn = ps.tile([128, RT, D], f32, tag=f"xnat{b}")
                for a in range(RT):
                    xn_ps = pmm.tile([128, D], f32, tag="mm")
                    nc.tensor.matmul(
                        out=xn_ps[:], lhsT=x_meanT[b][:, a * 128:(a + 1) * 128],
                        rhs=w_sb[:], start=True, stop=True,
                    )
                    nc.scalar.copy(xn[:, a, :], xn_ps[:])
                xnat.append(xn)
            nc.sync.dma_start(
                out=out_x0[:].rearrange("(a p) d -> p a d", p=128), in_=xnat[0][:]
            )
            nc.sync.dma_start(
                out=out_x1[:].rearrange("(a p) d -> p a d", p=128), in_=xnat[1][:]
            )

            # s_shard^T = col-sums of x0T_shard**2 -> row 64 of ut_sh
            sq_sh = wk.tile([D, SH], f32, tag="sq_sh")
            nc.vector.tensor_mul(sq_sh[:], ut_sh[0:D, :], ut_sh[0:D, :])
            s_ps = pmm.tile([1, SH], f32, tag="mm")
            nc.tensor.matmul(out=s_ps[:], lhsT=ones64[:], rhs=sq_sh[:],
                             start=True, stop=True)
            nc.scalar.copy(out=ut_sh[D:D + 1, :], in_=s_ps[:])

            # ---------- phase B: AllGather x0T+s and x0 ----------
            bounce65 = dram.tile([D + 1, SH], f32)
            gat65 = dram.tile([NC * (D + 1), SH], f32, addr_space="Shared")
            bounce_x0 = dram.tile([SH, D], f32)
            gat_x0 = dram.tile([N, D], f32, addr_space="Shared")
            nc.sync.dma_start(out=bounce65[0:D, :], in_=ut_sh[0:D, :])
            nc.sync.dma_start(out=bounce65[D:D + 1, :], in_=ut_sh[D:D + 1, :])
            nc.sync.dma_start(
                out=bounce_x0[:].rearrange("(a p) d -> p a d", p=128), in_=xnat[0][:]
            )
            nc.gpsimd.collective_compute(
                "AllGather", mybir.AluOpType.bypass,
                replica_groups=[list(range(NC))],
                ins=[bounce65.opt()], outs=[gat65.opt()],
            )
            nc.gpsimd.collective_compute(
                "AllGather", mybir.AluOpType.bypass,
                replica_groups=[list(range(NC))],
                ins=[bounce_x0.opt()], outs=[gat_x0.opt()],
            )

            # V'^T [CP, N]: 0..63 x0T, 64 s^T, 65 ones, 66..95 zero, 96 -r^T
            vpt = ps.tile([CP, N], f32)
            nc.vector.memset(vpt[D:96, :], 0.0)
            nc.vector.memset(vpt[96:CP, :], 0.0)
            nc.vector.memset(vpt[D:D + 2, :], 1.0)
            for c in range(NC):
                base = c * (D + 1)
                nc.sync.dma_start(
                    out=vpt[0:D, c * SH:(c + 1) * SH], in_=gat65[base:base + D, :]
                )
                nc.sync.dma_start(
                    out=vpt[D:D + 1, c * SH:(c + 1) * SH],
                    in_=gat65[base + D:base + D + 1, :],
                )

            # ---------- phase C: W~ = Gram(U~ H), r, U'^T ----------
            # chunk columns: 0..63 = -2*x0, 64 = ones, 65 = s, 66..96 = 0
            wt_ps = pw66.tile([CP, CP], f32)
            for ch in range(N // 128):
                vch = vchp.tile([128, CP], f32, tag="vch")
                nc.sync.dma_start(
                    out=vch[:, 0:D], in_=gat_x0[ch * 128:(ch + 1) * 128, :]
                )
                nc.vector.tensor_scalar_mul(vch[:, 0:D], vch[:, 0:D], -2.0)
                nc.vector.memset(vch[:, D + 2:], 0.0)
                nc.vector.memset(vch[:, D:D + 1], 1.0)
                c_blk, j0 = (ch * 128) // SH, (ch * 128) % SH
                nc.sync.dma_start(
                    out=vch[:, D + 1:D + 2],
                    in_=gat65[c_blk * (D + 1) + D, j0:j0 + 128].rearrange(
                        "(p o) -> p o", o=1
                    ),
                )
                nc.tensor.matmul(out=wt_ps[:], lhsT=vch[:], rhs=vch[:],
                                 start=(ch == 0), stop=(ch == N // 128 - 1))
            wt_sb = ps.tile([CP, CP], f32)
            nc.scalar.copy(out=wt_sb[:], in_=wt_ps[:])

            # r^T chunks: r = colsum(V~ * (W~ V~)) -> row 96 of vpt (negated)
            for ch in range(CH):
                sl = slice(ch * 512, (ch + 1) * 512)
                p_ps = pmm.tile([CP, 512], f32, tag="mm")
                nc.tensor.matmul(out=p_ps[:], lhsT=wt_sb[:], rhs=vpt[:, sl],
                                 start=True, stop=True)
                tmp = wk.tile([CP, 512], f32, tag="rtmp")
                nc.vector.tensor_mul(tmp[:], p_ps[:], vpt[:, sl])
                r_ps = pmm.tile([1, 512], f32, tag="mm")
                nc.tensor.matmul(out=r_ps[:], lhsT=ones97[:], rhs=tmp[:],
                                 start=True, stop=True)
                nc.scalar.mul(vpt[96:CP, sl], r_ps[:], -1.0)

            # U'^T [CP, SH] = [2 * W~ @ u~_shard^T ; row96 ones]
            upt = ps.tile([CP, SH], f32)
            up_ps = pmm.tile([CP, SH], f32, tag="mm")
            nc.tensor.matmul(out=up_ps[:], lhsT=wt_sb[:], rhs=ut_sh[:],
                             start=True, stop=True)
            nc.scalar.mul(upt[:], up_ps[:], 2.0)
            nc.vector.memset(upt[96:CP, :], 1.0)

            # ---------- phase D: A = U'^T.T @ V'^T, top-8, gather, logprobs ----
            for rt in range(RT):
                a_sb = abp.tile([128, N], f32, tag="a_sb")
                for ch in range(CH):
                    a_ps = pa.tile([128, 512], f32, tag="a_ps")
                    nc.tensor.matmul(
                        out=a_ps[:], lhsT=upt[:, rt * 128:(rt + 1) * 128],
                        rhs=vpt[:, ch * 512:(ch + 1) * 512], start=True, stop=True,
                    )
                    nc.scalar.copy(out=a_sb[:, ch * 512:(ch + 1) * 512], in_=a_ps[:])
                mx = wk.tile([128, K], f32, tag="mx")
                idx = wk.tile([128, K], u32, tag="idx")
                nc.vector.max(out=mx[:], in_=a_sb[:])
                nc.vector.max_index(out=idx[:], in_max=mx[:], in_values=a_sb[:])
                nc.sync.dma_start(out=out_idx[rt * 128:(rt + 1) * 128, :], in_=idx[:])

                # gather neighbor embeddings, logprobs = -temp * ||x_i - x_j||^2
                g = wk.tile([128, K, D], f32, tag="g")
                for m_ in range(K):
                    nc.gpsimd.indirect_dma_start(
                        out=g[:, m_, :], out_offset=None, in_=gat_x0[:],
                        in_offset=bass.IndirectOffsetOnAxis(ap=idx[:, m_:m_ + 1], axis=0),
                    )
                dif = wk.tile([128, K, D], f32, tag="dif")
                nc.vector.tensor_sub(
                    dif[:], g[:],
                    xnat[0][:, rt, :].rearrange("p (o d) -> p o d", o=1)
                    .broadcast_to((128, K, D)),
                )
                sq_scr = wk.tile([128, K, D], f32, tag="sq_scr")
                nc.vector.tensor_mul(sq_scr[:], dif[:], dif[:])
                ssd = wk.tile([128, K], f32, tag="ssd")
                nc.vector.reduce_sum(out=ssd[:], in_=sq_scr[:],
                                     axis=mybir.AxisListType.X)
                lp = wk.tile([128, K], f32, tag="lp")
                nc.scalar.mul(lp[:], ssd[:], -float(temp))
                nc.sync.dma_start(out=out_logp[rt * 128:(rt + 1) * 128, :], in_=lp[:])

    nc.compile()
    return nc


@functools.lru_cache(maxsize=2)
def _built(temp: float):
    return build_nc(temp)


def _in_maps(x_pre, W_embed):
    x_pre = np.ascontiguousarray(x_pre, dtype=np.float32)
    w = np.ascontiguousarray(W_embed, dtype=np.float32)
    ones = np.ones(N, dtype=np.float32)
    maps = []
    for c in range(NC):
        maps.append({
            "xp": np.ascontiguousarray(x_pre[:, :, c * SH:(c + 1) * SH, :]),
            "w_in": w,
            "cones": ones,
        })
    return maps


def _assemble(results, temp):
    x0 = np.concatenate([r["out_x0"] for r in results], axis=0)
    x1 = np.concatenate([r["out_x1"] for r in results], axis=0)
    idx = np.concatenate([r["out_idx"] for r in results], axis=0).astype(np.int32)
    logp = np.concatenate([r["out_logp"] for r in results], axis=0)
    x = np.stack([x0, x1], axis=0)
    src = np.repeat(np.arange(N, dtype=np.int32), K)
    edges = np.stack([src, idx.reshape(-1)], axis=0)
    logprobs = logp.reshape(1, N, K)
    return x, edges, logprobs


def kernel(x_pre, W_embed, temperature, k):
    assert int(k) == K
    t = np.clip(np.asarray(temperature, dtype=np.float32).reshape(-1)[0],
                np.float32(-5.0), np.float32(5.0))
    temp = float(np.exp(t, dtype=np.float32))
    nc = _built(temp)
    res = bass_utils.run_bass_kernel_spmd(nc, _in_maps(x_pre, W_embed),
                                          core_ids=list(range(NC)))
    return _assemble(res.results, temp)


# revision 11
# speedup vs baseline: 1.3917x; 1.3917x over previous
"""Trainium2 Bass kernel for nn_DGM_d (retrieval_knn).

Model: x = einsum(mean_t(x_pre), W); lq = pairwise_sq_dists(x[0]) * temp;
D2 = pairwise_sq_dists(lq); idx = top_k(-D2, 8); gather + logprobs; edges.

Device strategy (8 cores, query-row sharded):
  With U~ = [x0, s, 1] (n x 66) and the mixing matrix H (x0 block -> -2*I,
  s <-> 1 swap), lq = U~ H U~^T exactly (rank 66). The D2 row-ranking then
  reduces to A = 2*U~ W~ U~^T - 1 r^T with W~ = H (U~^T U~) H = Gram(U~ H)
  (66x66) and r_j = u~_j^T W~ u~_j — a 97-wide padded contraction instead
  of the naive 4096-wide one (~60x fewer flops).

  Component layout on partitions (legal partial-access starts 0/32/64/96):
  rows 0..63 x0^T, row 64 s, row 65 ones, rows 66..95 zero, row 96 -r.

  Each core computes A for its 512 query rows against all 4096 columns and
  takes top-8 per row with the HW max8/max_index instructions (semantics
  match jax.lax.top_k incl. tie order). Embeddings are computed on device
  (mean via DVE reduce + PE transpose + matmul); x0 shards are AllGathered.
  logprobs come from gathered neighbor embeddings (indirect DMA) so
  self-pairs give exactly 0, matching the reference's direct formula.
"""

import functools

import numpy as np

import concourse.bacc as bacc
import concourse.bass as bass
import concourse.mybir as mybir
from concourse import bass_utils
from concourse.masks import make_identity
from concourse.tile import TileContext

N = 4096          # nodes
D = 64            # embed dim
F = 32            # input feature dim
T = 24            # temporal dim
K = 8             # neighbors
NC = 8            # cores
SH = N // NC      # 512 rows per core
CP = 97           # padded contract dim: x0(64) + s + ones + pad(30) + r
RT = SH // 128    # 4 row-tiles per core
CH = N // 512     # 8 column chunks of 512
f32 = mybir.dt.float32
u32 = mybir.dt.uint32


def build_nc(temp: float):
    nc = bacc.Bacc("TRN2", target_bir_lowering=False, debug=False, num_devices=NC)

    xp = nc.dram_tensor("xp", [2, T, SH, F], f32, kind="ExternalInput")
    w_in = nc.dram_tensor("w_in", [F, D], f32, kind="ExternalInput")
    cones = nc.dram_tensor("cones", [N], f32, kind="ExternalInput")
    out_x0 = nc.dram_tensor("out_x0", [SH, D], f32, kind="ExternalOutput")
    out_x1 = nc.dram_tensor("out_x1", [SH, D], f32, kind="ExternalOutput")
    out_idx = nc.dram_tensor("out_idx", [SH, K], u32, kind="ExternalOutput")
    out_logp = nc.dram_tensor("out_logp", [SH, K], f32, kind="ExternalOutput")

    with TileContext(nc) as tc:
        with (
            tc.tile_pool(name="dram", bufs=1, space="DRAM") as dram,
            tc.tile_pool(name="persist", bufs=1) as ps,
            tc.tile_pool(name="work", bufs=2) as wk,
            tc.tile_pool(name="vch", bufs=3) as vchp,
            tc.tile_pool(name="abuf", bufs=2) as abp,
            tc.tile_pool(name="pp_mm", bufs=2, space="PSUM") as pmm,
            tc.tile_pool(name="pp_w66", bufs=1, space="PSUM") as pw66,
            tc.tile_pool(name="pp_tr", bufs=2, space="PSUM") as ptr,
            tc.tile_pool(name="pp_a", bufs=3, space="PSUM") as pa,
        ):
            # ---------- constants ----------
            ident = ps.tile([128, 128], f32)
            make_identity(nc, ident[:])
            w_sb = ps.tile([F, D], f32)
            nc.sync.dma_start(out=w_sb[:], in_=w_in[:])
            ones64 = ps.tile([D, 1], f32)
            nc.sync.dma_start(out=ones64[:], in_=cones[:D].rearrange("(p o) -> p o", o=1))
            ones97 = ps.tile([CP, 1], f32)
            nc.sync.dma_start(out=ones97[:], in_=cones[:CP].rearrange("(p o) -> p o", o=1))

            # ---------- phase A: load shard, mean over t, embed ----------
            # SBUF layout [p = n//4, t, q = n%4, f]; DRAM runs of 512B.
            x_meanT = []  # [F, SH] per batch: transposed means
            for b in range(2):
                ld = wk.tile([128, T, 4, F], f32, tag="ld")
                nc.sync.dma_start(
                    out=ld[:], in_=xp[b].rearrange("t (p q) f -> p t q f", q=4)
                )
                m = wk.tile([128, 4, F], f32, tag="m")
                nc.vector.reduce_sum(
                    out=m[:], in_=ld[:].rearrange("p t q f -> p q f t"),
                    axis=mybir.AxisListType.X,
                )
                mt_ps = ptr.tile([128, 128], f32)
                nc.tensor.transpose(
                    out=mt_ps[:], in_=m[:].rearrange("p q f -> p (q f)"), identity=ident[:]
                )
                xmt = wk.tile([F, SH], f32, tag="xmt")
                xmt_v = xmt[:].rearrange("f (n q) -> f n q", q=4)
                for q in range(4):
                    # scale by 1/T to turn the t-sum into the mean
                    nc.scalar.mul(xmt_v[:, :, q], mt_ps[q * F:(q + 1) * F, :], 1.0 / T)
                x_meanT.append(xmt)

            # u~_shard^T [CP, SH]: rows 0..63 x0T_shard, 64 s, 65 ones, rest 0
            ut_sh = ps.tile([CP, SH], f32)
            nc.vector.memset(ut_sh[D:96, :], 0.0)
            nc.vector.memset(ut_sh[D:D + 2, :], 1.0)  # row64 (s, for now) + row65
            nc.vector.memset(ut_sh[96:CP, :], 0.0)
            x0t_ps = pmm.tile([D, SH], f32, tag="mm")
            nc.tensor.matmul(out=x0t_ps[:], lhsT=w_sb[:], rhs=x_meanT[0][:],
                             start=True, stop=True)
            nc.scalar.copy(out=ut_sh[0:D, :], in_=x0t_ps[:])

            # natural-layout embeds for both batches: [128, RT, D]
            xnat = []
            for b in range(2):
                xn = ps.tile([128, RT, D], f32, tag=f"xnat{b}")
                for a in range(RT):
                    xn_ps = pmm.tile([128, D], f32, tag="mm")
                    nc.tensor.matmul(
                        out=xn_ps[:], lhsT=x_meanT[b][:, a * 128:(a + 1) * 128],
                        rhs=w_sb[:], start=True, stop=True,
                    )
                    nc.scalar.copy(xn[:, a, :], xn_ps[:])
                xnat.append(xn)
            nc.sync.dma_start(
                out=out_x0[:].rearrange("(a p) d -> p a d", p=128), in_=xnat[0][:]
            )
            nc.sync.dma_start(
                out=out_x1[:].rearrange("(a p) d -> p a d", p=128), in_=xnat[1][:]
            )

            # s_shard^T = col-sums of x0T_shard**2 -> row 64 of ut_sh
            sq_sh = wk.tile([D, SH], f32, tag="sq_sh")
            nc.vector.tensor_mul(sq_sh[:], ut_sh[0:D, :], ut_sh[0:D, :])
            s_ps = pmm.tile([1, SH], f32, tag="mm")
            nc.tensor.matmul(out=s_ps[:], lhsT=ones64[:], rhs=sq_sh[:],
                             start=True, stop=True)
            nc.scalar.copy(out=ut_sh[D:D + 1, :], in_=s_ps[:])

            # ---------- phase B: AllGather x0T+s and x0 ----------
            bounce65 = dram.tile([D + 1, SH], f32)
            gat65 = dram.tile([NC * (D + 1), SH], f32, addr_space="Shared")
            bounce_x0 = dram.tile([SH, D], f32)
            gat_x0 = dram.tile([N, D], f32, addr_space="Shared")
            nc.sync.dma_start(out=bounce65[0:D, :], in_=ut_sh[0:D, :])
            nc.sync.dma_start(out=bounce65[D:D + 1, :], in_=ut_sh[D:D + 1, :])
            nc.sync.dma_start(
                out=bounce_x0[:].rearrange("(a p) d -> p a d", p=128), in_=xnat[0][:]
            )
            nc.gpsimd.collective_compute(
                "AllGather", mybir.AluOpType.bypass,
                replica_groups=[list(range(NC))],
                ins=[bounce65.opt()], outs=[gat65.opt()],
            )
            nc.gpsimd.collective_compute(
                "AllGather", mybir.AluOpType.bypass,
                replica_groups=[list(range(NC))],
                ins=[bounce_x0.opt()], outs=[gat_x0.opt()],
            )

            # V'^T [CP, N]: 0..63 x0T, 64 s^T, 65 ones, 66..95 zero, 96 -r^T
            vpt = ps.tile([CP, N], f32)
            nc.vector.memset(vpt[D:96, :], 0.0)
            nc.vector.memset(vpt[96:CP, :], 0.0)
            nc.vector.memset(vpt[D:D + 2, :], 1.0)
            for c in range(NC):
                base = c * (D + 1)
                nc.sync.dma_start(
                    out=vpt[0:D, c * SH:(c + 1) * SH], in_=gat65[base:base + D, :]
                )
                nc.sync.dma_start(
                    out=vpt[D:D + 1, c * SH:(c + 1) * SH],
                    in_=gat65[base + D:base + D + 1, :],
                )

            # ---------- phase C: W~ = Gram(U~ H), r, U'^T ----------
            # chunk columns: 0..63 = -2*x0, 64 = ones, 65 = s, 66..96 = 0
            wt_ps = pw66.tile([CP, CP], f32)
            for ch in range(N // 128):
                vch = vchp.tile([128, CP], f32, tag="vch")
                nc.sync.dma_start(
                    out=vch[:, 0:D], in_=gat_x0[ch * 128:(ch + 1) * 128, :]
                )
                nc.vector.tensor_scalar_mul(vch[:, 0:D], vch[:, 0:D], -2.0)
                nc.vector.memset(vch[:, D + 2:], 0.0)
                nc.vector.memset(vch[:, D:D + 1], 1.0)
                c_blk, j0 = (ch * 128) // SH, (ch * 128) % SH
                nc.sync.dma_start(
                    out=vch[:, D + 1:D + 2],
                    in_=gat65[c_blk * (D + 1) + D, j0:j0 + 128].rearrange(
                        "(p o) -> p o", o=1
                    ),
                )
                nc.tensor.matmul(out=wt_ps[:], lhsT=vch[:], rhs=vch[:],
                                 start=(ch == 0), stop=(ch == N // 128 - 1))
            wt_sb = ps.tile([CP, CP], f32)
            nc.scalar.copy(out=wt_sb[:], in_=wt_ps[:])

            # r^T chunks: r = colsum(V~ * (W~ V~)) -> row 96 of vpt (negated)
            for ch in range(CH):
                sl = slice(ch * 512, (ch + 1) * 512)
                p_ps = pmm.tile([CP, 512], f32, tag="mm")
                nc.tensor.matmul(out=p_ps[:], lhsT=wt_sb[:], rhs=vpt[:, sl],
                                 start=True, stop=True)
                tmp = wk.tile([CP, 512], f32, tag="rtmp")
                nc.vector.tensor_mul(tmp[:], p_ps[:], vpt[:, sl])
                r_ps = pmm.tile([1, 512], f32, tag="mm")
                nc.tensor.matmul(out=r_ps[:], lhsT=ones97[:], rhs=tmp[:],
                                 start=True, stop=True)
                nc.scalar.mul(vpt[96:CP, sl], r_ps[:], -1.0)

            # U'^T [CP, SH] = [2 * W~ @ u~_shard^T ; row96 ones]
            upt = ps.tile([CP, SH], f32)
            up_ps = pmm.tile([CP, SH], f32, tag="mm")
            nc.tensor.matmul(out=up_ps[:], lhsT=wt_sb[:], rhs=ut_sh[:],
                             start=True, stop=True)
            nc.scalar.mul(upt[:], up_ps[:], 2.0)
            nc.vector.memset(upt[96:CP, :], 1.0)

            # ---------- phase D: A = U'^T.T @ V'^T, top-8, gather, logprobs ----
            for rt in range(RT):
                a_sb = abp.tile([128, N], f32, tag="a_sb")
                for ch in range(CH):
                    a_ps = pa.tile([128, 512], f32, tag="a_ps")
                    nc.tensor.matmul(
                        out=a_ps[:], lhsT=upt[:, rt * 128:(rt + 1) * 128],
                        rhs=vpt[:, ch * 512:(ch + 1) * 512], start=True, stop=True,
                    )
                    nc.scalar.copy(out=a_sb[:, ch * 512:(ch + 1) * 512], in_=a_ps[:])
                mx = wk.tile([128, K], f32, tag="mx")
                idx = wk.tile([128, K], u32, tag="idx")
                nc.vector.max(out=mx[:], in_=a_sb[:])
                nc.vector.max_index(out=idx[:], in_max=mx[:], in_values=a_sb[:])
                nc.sync.dma_start(out=out_idx[rt * 128:(rt + 1) * 128, :], in_=idx[:])

                # gather neighbor embeddings, logprobs = -temp * ||x_i - x_j||^2
                g = wk.tile([128, K, D], f32, tag="g")
                for m_ in range(K):
                    nc.gpsimd.indirect_dma_start(
                        out=g[:, m_, :], out_offset=None, in_=gat_x0[:],
                        in_offset=bass.IndirectOffsetOnAxis(ap=idx[:, m_:m_ + 1], axis=0),
                    )
                dif = wk.tile([128, K, D], f32, tag="dif")
                nc.vector.tensor_sub(
                    dif[:], g[:],
                    xnat[0][:, rt, :].rearrange("p (o d) -> p o d", o=1)
                    .broadcast_to((128, K, D)),
                )
                sq_scr = wk.tile([128, K, D], f32, tag="sq_scr")
                nc.vector.tensor_mul(sq_scr[:], dif[:], dif[:])
                ssd = wk.tile([128, K], f32, tag="ssd")
                nc.vector.reduce_sum(out=ssd[:], in_=sq_scr[:],
                                     axis=mybir.AxisListType.X)
                lp = wk.tile([128, K], f32, tag="lp")
                nc.scalar.mul(lp[:], ssd[:], -float(temp))
                nc.sync.dma_start(out=out_logp[rt * 128:(rt + 1) * 128, :], in_=lp[:])

    nc.compile()
    return nc


@functools.lru_cache(maxsize=2)
def _built(temp: float):
    return build_nc(temp)


@functools.lru_cache(maxsize=2)
def _runner(temp: float):
    """Build a persistent jitted PJRT executable for the compiled module.

    Mirrors bass2jax.run_bass_via_pjrt's multi-core path but caches the
    jitted callable so repeated kernel() calls skip retrace/recompile.
    """
    import jax
    from jax.sharding import Mesh, PartitionSpec
    from jax.experimental.shard_map import shard_map
    from concourse import bass2jax, mybir as mb

    nc = _built(temp)
    bass2jax.install_neuronx_cc_hook()
    partition_name = nc.partition_id_tensor.name if nc.partition_id_tensor else None

    in_names, out_names, out_avals = [], [], []
    for alloc in nc.m.functions[0].allocations:
        if not isinstance(alloc, mb.MemoryLocationSet):
            continue
        name = alloc.memorylocations[0].name
        if alloc.kind == "ExternalInput":
            if name != partition_name:
                in_names.append(name)
        elif alloc.kind == "ExternalOutput":
            out_names.append(name)
            out_avals.append(
                jax.core.ShapedArray(tuple(alloc.tensor_shape), mb.dt.np(alloc.dtype))
            )
    n_params = len(in_names)
    all_in_names = list(in_names) + list(out_names)
    if partition_name is not None:
        all_in_names.append(partition_name)

    def _body(*args):
        operands = list(args)
        if partition_name is not None:
            operands.append(bass2jax.partition_id_tensor())
        return tuple(bass2jax._bass_exec_p.bind(
            *operands,
            out_avals=tuple(out_avals),
            in_names=tuple(all_in_names),
            out_names=tuple(out_names),
            lowering_input_output_aliases=(),
            sim_require_finite=True,
            sim_require_nnan=True,
            nc=nc,
        ))

    devices = jax.devices()[:NC]
    mesh = Mesh(np.asarray(devices), ("core",))
    n_outs = len(out_names)
    sharded = jax.jit(
        shard_map(
            _body, mesh=mesh,
            in_specs=(PartitionSpec("core"),) * (n_params + n_outs),
            out_specs=(PartitionSpec("core"),) * n_outs,
            check_rep=False,
        ),
        donate_argnums=tuple(range(n_params, n_params + n_outs)),
        keep_unused=True,
    )

    zero_templates = [
        (tuple(a.shape), a.dtype) for a in out_avals
    ]

    def run(in_maps):
        concat_in = [
            np.concatenate([np.asarray(in_maps[c][nm]) for c in range(NC)], axis=0)
            for nm in in_names
        ]
        zeros = [np.zeros((NC * s[0], *s[1:]), dt) for s, dt in zero_templates]
        outs = sharded(*concat_in, *zeros)
        return [
            {nm: np.asarray(outs[i]).reshape(NC, *out_avals[i].shape)[c]
             for i, nm in enumerate(out_names)}
            for c in range(NC)
        ]

    return run


def _in_maps(x_pre, W_embed):
    x_pre = np.ascontiguousarray(x_pre, dtype=np.float32)
    w = np.ascontiguousarray(W_embed, dtype=np.float32)
    ones = np.ones(N, dtype=np.float32)
    maps = []
    for c in range(NC):
        maps.append({
            "xp": np.ascontiguousarray(x_pre[:, :, c * SH:(c + 1) * SH, :]),
            "w_in": w,
            "cones": ones,
        })
    return maps


def _assemble(results, temp):
    x0 = np.concatenate([r["out_x0"] for r in results], axis=0)
    x1 = np.concatenate([r["out_x1"] for r in results], axis=0)
    idx = np.concatenate([r["out_idx"] for r in results], axis=0).astype(np.int32)
    logp = np.concatenate([r["out_logp"] for r in results], axis=0)
    x = np.stack([x0, x1], axis=0)
    src = np.repeat(np.arange(N, dtype=np.int32), K)
    edges = np.stack([src, idx.reshape(-1)], axis=0)
    logprobs = logp.reshape(1, N, K)
    return x, edges, logprobs


def kernel(x_pre, W_embed, temperature, k):
    assert int(k) == K
    t = np.clip(np.asarray(temperature, dtype=np.float32).reshape(-1)[0],
                np.float32(-5.0), np.float32(5.0))
    temp = float(np.exp(t, dtype=np.float32))
    results = _runner(temp)(_in_maps(x_pre, W_embed))
    return _assemble(results, temp)
